# revision 1
# baseline (speedup 1.0000x reference)
"""MiniMoE (T=8192, D=1024, E=8, K=2) — expert-parallel Bass kernel for 8 trn2 NeuronCores.

Strategy: each core owns one expert. The host gathers the tokens routed to each
expert (transposed to [D, C] so every device DMA is contiguous), each core runs
relu(relu(x @ W1.T) @ W2.T) for its expert's tokens only (4x less compute than
the dense reference), and the host scatters the per-expert outputs back with the
routing weights.

Matmuls run as float32r (full-rate fp32 streaming mode on the PE array).
"""

import os
import sys

sys.path.insert(0, "/opt/trn_rl_repo")

import numpy as np

T, D = 8192, 1024
E, K = 8, 2
NCORES = 8
P = 128
TOK_TILE = 512
ND = D // P  # 8 feature tiles

_kernel_cache: dict = {}


def _build_bass(C: int, io_bf16: bool):
    """Build + compile the per-core Bass program for token capacity C (multiple of 128).

    io_bf16=True transports xt/w1t/w2t as bf16 (half the HBM traffic) and
    upconverts on-chip (DVE) to float32r before the matmuls.
    """
    import concourse.bacc as bacc
    import concourse.mybir as mybir
    from concourse import tile

    f32 = mybir.dt.float32
    f32r = mybir.dt.float32r
    bf16 = mybir.dt.bfloat16
    io_dt = bf16 if io_bf16 else f32r
    Relu = mybir.ActivationFunctionType.Relu

    nc = bacc.Bacc(None, target_bir_lowering=False, debug=False)

    with tile.TileContext(nc) as tc:
        xt = nc.dram_tensor("xt", [D, C], io_dt, kind="ExternalInput")
        w1t = nc.dram_tensor("w1t", [D, D], io_dt, kind="ExternalInput")
        w2t = nc.dram_tensor("w2t", [D, D], io_dt, kind="ExternalInput")
        yt = nc.dram_tensor("yt", [D, C], f32, kind="ExternalOutput")

        import contextlib
        with contextlib.ExitStack() as _stk:
            wpool = _stk.enter_context(tc.tile_pool(name="wpool", bufs=1))
            apool = _stk.enter_context(tc.tile_pool(name="apool", bufs=2))
            hpool = _stk.enter_context(tc.tile_pool(name="hpool", bufs=4))
            spool = _stk.enter_context(tc.tile_pool(name="spool", bufs=4)) if io_bf16 else None
            opool = _stk.enter_context(tc.tile_pool(name="opool", bufs=3))
            ppool = _stk.enter_context(tc.tile_pool(name="ppool", bufs=8, space="PSUM"))

            # Weights resident as 8 stacked [128, 1024] row-blocks. DMA issue
            # is ~0.6us per dma_start per engine queue, so loads are spread
            # across engine queues (w1/w2 -> sync, xt -> gpsimd, stores ->
            # scalar) and ordered so the head of the pipeline (layer 1 of the
            # first token tile) gets its inputs first.
            w1_sb = wpool.tile([P, ND * D], f32r, tag="w1sb")
            w2_sb = wpool.tile([P, ND * D], f32r, tag="w2sb")
            n0 = min(TOK_TILE, C)
            ntile = (C + TOK_TILE - 1) // TOK_TILE

            # PE clock warmup: the HAM throttles a cold PE to 1.2 GHz until it
            # has been busy ~3.4us. These dummy matmuls have no DMA inputs, so
            # they run right after the startup barrier and un-throttle the PE
            # before the first real matmul's data lands (~12us in).
            warm_src = opool.tile([P, P], f32, tag="warm")
            nc.gpsimd.memset(warm_src[:], 0.0)
            warm_ps = ppool.tile([P, TOK_TILE], f32, tag="ps", name="warm_ps")
            for _ in range(14):
                nc.tensor.matmul(warm_ps[:, :P], lhsT=warm_src[:],
                                 rhs=warm_src[:], start=True, stop=True)

            def load_block(engine, dst, src, stage_tag):
                """DMA a block (optionally via bf16 staging + DVE upconvert)."""
                if io_bf16:
                    stg = spool.tile([P, src.shape[1]], bf16, tag=stage_tag,
                                     name=f"{stage_tag}_{len(nc.m.functions[0].allocations)}")
                    engine.dma_start(out=stg[:, :], in_=src)
                    nc.vector.tensor_copy(dst, stg[:, :])
                else:
                    engine.dma_start(out=dst, in_=src)

            # All input DMAs ride the sync queue in strict need-order —
            # one queue-set avoids cross-stream bandwidth competition, and the
            # phase split (all of layer 1 first) means w2 is needed LAST:
            #   w1/xt(j0) -> xt(j1) -> xt(j2) -> xt(j3) -> w2.
            xt_sbs = [None] * ntile
            xt_sbs[0] = apool.tile([P, ND * TOK_TILE], f32r, tag="xt", name="xt_0")
            if not io_bf16:
                nc.sync.dma_start(out=w1_sb[:, 0:D // 2], in_=w1t[0:P, 0:D // 2])
            for d in range(ND):
                load_block(nc.sync,
                           xt_sbs[0][:, d * TOK_TILE: d * TOK_TILE + n0],
                           xt[d * P:(d + 1) * P, 0:n0], "xstage")
                if d == 0 and not io_bf16:
                    nc.sync.dma_start(out=w1_sb[:, D // 2:D], in_=w1t[0:P, D // 2:D])
                elif d == 0:
                    load_block(nc.sync, w1_sb[:, 0:D], w1t[0:P, :], "wstage")
                else:
                    load_block(nc.sync,
                               w1_sb[:, d * D:(d + 1) * D],
                               w1t[d * P:(d + 1) * P, :], "wstage")
            for j in range(1, ntile):
                n = min(TOK_TILE, C - j * TOK_TILE)
                xt_sbs[j] = apool.tile([P, ND * TOK_TILE], f32r, tag="xt",
                                       name=f"xt_{j}")
                for d in range(ND):
                    load_block(nc.sync,
                               xt_sbs[j][:, d * TOK_TILE: d * TOK_TILE + n],
                               xt[d * P:(d + 1) * P,
                                  j * TOK_TILE: j * TOK_TILE + n], "xstage")
            for d in range(ND):
                load_block(nc.sync,
                           w2_sb[:, d * D:(d + 1) * D],
                           w2t[d * P:(d + 1) * P, :], "wstage")

            # Phase 1 — layer 1 for every token tile (consumes only w1 + xt).
            # j=0 runs contraction-major (d outer, 8 PSUM groups in flight) so
            # the PE starts as soon as the first w1/xt blocks land and trickles
            # at DMA rate; later tiles run o-major so relu evictions pipeline.
            ht_sbs = []
            for j in range(ntile):
                n = min(TOK_TILE, C - j * TOK_TILE)
                xt_sb = xt_sbs[j]
                ht_sb = hpool.tile([P, ND * TOK_TILE], f32r, tag="ht",
                                   name=f"ht_{j}")
                ht_sbs.append(ht_sb)
                if j == 0:
                    pss = [ppool.tile([P, TOK_TILE], f32, tag="ps", name=f"ps0_{o}")
                           for o in range(ND)]
                    for d in range(ND):
                        for o in range(ND):
                            nc.tensor.matmul(
                                pss[o][:, :n],
                                lhsT=w1_sb[:, d * D + o * P: d * D + (o + 1) * P],
                                rhs=xt_sb[:, d * TOK_TILE: d * TOK_TILE + n],
                                start=(d == 0), stop=(d == ND - 1))
                    for o in range(ND):
                        nc.scalar.activation(
                            ht_sb[:, o * TOK_TILE: o * TOK_TILE + n],
                            pss[o][:, :n], Relu)
                else:
                    for o in range(ND):
                        ps = ppool.tile([P, TOK_TILE], f32, tag="ps")
                        for d in range(ND):
                            nc.tensor.matmul(
                                ps[:, :n],
                                lhsT=w1_sb[:, d * D + o * P: d * D + (o + 1) * P],
                                rhs=xt_sb[:, d * TOK_TILE: d * TOK_TILE + n],
                                start=(d == 0), stop=(d == ND - 1))
                        nc.scalar.activation(
                            ht_sb[:, o * TOK_TILE: o * TOK_TILE + n], ps[:, :n], Relu)

            # Phase 2 — layer 2 for every token tile (w2 is long resident).
            for j in range(ntile):
                n = min(TOK_TILE, C - j * TOK_TILE)
                ht_sb = ht_sbs[j]
                for p_ in range(ND):
                    ps2 = ppool.tile([P, TOK_TILE], f32, tag="ps")
                    for o in range(ND):
                        nc.tensor.matmul(
                            ps2[:, :n],
                            lhsT=w2_sb[:, o * D + p_ * P: o * D + (p_ + 1) * P],
                            rhs=ht_sb[:, o * TOK_TILE: o * TOK_TILE + n],
                            start=(o == 0), stop=(o == ND - 1))
                    yo = opool.tile([P, TOK_TILE], f32, tag="yo")
                    nc.scalar.activation(yo[:, :n], ps2[:, :n], Relu)
                    nc.sync.dma_start(
                        out=yt[p_ * P:(p_ + 1) * P, j * TOK_TILE: j * TOK_TILE + n],
                        in_=yo[:, :n])

    nc.compile()
    return nc


def _get_bass(C: int, io_bf16: bool):
    key = (C, io_bf16)
    if key not in _kernel_cache:
        _kernel_cache[key] = _build_bass(C, io_bf16)
    return _kernel_cache[key]


LAST_RESULTS = None  # BassKernelResults of the most recent run (for test harness)


def kernel(x, flat_expert_indices, flat_expert_weights, W1, W2):
    global LAST_RESULTS
    from concourse.bass_utils import run_bass_kernel_spmd

    x = np.ascontiguousarray(np.asarray(x, dtype=np.float32))
    idx = np.asarray(flat_expert_indices).astype(np.int64)
    w = np.asarray(flat_expert_weights, dtype=np.float32)
    W1 = np.asarray(W1, dtype=np.float32)
    W2 = np.asarray(W2, dtype=np.float32)

    order = np.argsort(idx, kind="stable")
    counts = np.bincount(idx, minlength=E)
    starts = np.zeros(E + 1, dtype=np.int64)
    starts[1:] = np.cumsum(counts)

    # Device capacity per expert: T*K/E (perfectly balanced) — the handful of
    # token-pairs routed beyond it (random-routing overflow) are computed on
    # the host. For the target distribution this is <0.5% of the work.
    cap_max = (T * K) // E
    C = int(max(TOK_TILE, min(cap_max, ((counts.max() + P - 1) // P) * P)))
    io_bf16 = bool(os.environ.get("MOE_BF16_IO"))
    nc = _get_bass(C, io_bf16)

    in_maps = []
    pos_list = []
    over_list = []
    for e in range(E):
        pos = order[starts[e]:starts[e + 1]]
        pos_list.append(pos[:C])
        over_list.append(pos[C:])
        toks = pos[:C] // K
        xt = np.zeros((D, C), dtype=np.float32)
        if len(toks):
            xt[:, :len(toks)] = x[toks].T
        w1te = np.ascontiguousarray(W1[e].T)
        w2te = np.ascontiguousarray(W2[e].T)
        if io_bf16:
            import ml_dtypes
            bf = ml_dtypes.bfloat16
            xt, w1te, w2te = xt.astype(bf), w1te.astype(bf), w2te.astype(bf)
        in_maps.append({"xt": xt, "w1t": w1te, "w2t": w2te})

    trace = bool(os.environ.get("MOE_TRACE"))
    try:
        res = run_bass_kernel_spmd(
            nc, in_maps, list(range(NCORES)),
            trace=trace,
            trace_cores=(list(range(NCORES)) if os.environ.get("MOE_TRACE_MULTI") else [0]) if trace else None,
        )
    except Exception:
        if os.environ.get("MOE_TRACE_STRICT"):
            raise
        # Trace/profiling plumbing can be absent in some environments —
        # fall back to a plain (untraced) run rather than failing.
        prev = os.environ.get("BASS_NEVER_TRACE")
        os.environ["BASS_NEVER_TRACE"] = "1"
        try:
            res = run_bass_kernel_spmd(nc, in_maps, list(range(NCORES)))
        finally:
            if prev is None:
                os.environ.pop("BASS_NEVER_TRACE", None)
            else:
                os.environ["BASS_NEVER_TRACE"] = prev
    LAST_RESULTS = res

    out_flat = np.zeros((T * K, D), dtype=np.float32)
    for e in range(E):
        pos = pos_list[e]
        if len(pos):
            y = res.results[e]["yt"][:, :len(pos)].T  # [n_e, D]
            out_flat[pos] = y * w[pos][:, None]
        over = over_list[e]
        if len(over):
            h = np.maximum(x[over // K] @ W1[e].T, 0.0)
            y = np.maximum(h @ W2[e].T, 0.0)
            out_flat[over] = y * w[over][:, None]

    out = out_flat.reshape(T, K, D)
    return (out[:, 0, :] + out[:, 1, :]).astype(np.float32)



# revision 5
# speedup vs baseline: 1.1154x; 1.1154x over previous
"""MiniMoE (T=8192, D=1024, E=8, K=2) — expert-parallel Bass kernel for 8 trn2 NeuronCores.

Strategy: each core owns one expert. The host dedups (token, expert) pairs
(a token routed to the same expert in both top-k slots is computed once with
the combined routing weight — ~6% of the pairs for uniform routing), gathers
each expert's tokens transposed to [D, C] so every device DMA is contiguous,
and each core runs relu(relu(x @ W1.T) @ W2.T) for its expert's tokens only.

The whole device pipeline is bf16 (matmul operands, relu outputs, HBM IO):
the PE runs bf16 at the same 1 column/cycle rate as float32r but LDWEIGHTS
is cheaper and HBM traffic halves. fp8 would double PE throughput but fails
the accuracy budget (measured 5.5e-2 vs the 2e-2 gate).
"""

import os
import sys

sys.path.insert(0, "/opt/trn_rl_repo")

import numpy as np

T, D = 8192, 1024
E, K = 8, 2
NCORES = 8
P = 128
TOK_TILE = 512
ND = D // P  # 8 feature tiles
CAP = 1920   # balanced capacity: unique pairs / E = 15368/8 -> 15*128

_kernel_cache: dict = {}


def _build_bass(C: int):
    """Build + compile the per-core Bass program for token capacity C
    (multiple of 128). All tensors bf16; PSUM accumulates fp32."""
    import concourse.bacc as bacc
    import concourse.mybir as mybir
    from concourse import tile

    f32 = mybir.dt.float32
    bf16 = mybir.dt.bfloat16
    Relu = mybir.ActivationFunctionType.Relu

    nc = bacc.Bacc(None, target_bir_lowering=False, debug=False)

    ntile = (C + TOK_TILE - 1) // TOK_TILE
    sizes = [min(TOK_TILE, C - j * TOK_TILE) for j in range(ntile)]
    n0 = sizes[0]
    rest = C - n0  # tokens in tiles j>=1, loaded as one [128, rest] block per d

    with tile.TileContext(nc) as tc:
        xt = nc.dram_tensor("xt", [D, C], bf16, kind="ExternalInput")
        w1t = nc.dram_tensor("w1t", [D, D], bf16, kind="ExternalInput")
        w2t = nc.dram_tensor("w2t", [D, D], bf16, kind="ExternalInput")
        yt = nc.dram_tensor("yt", [D, C], bf16, kind="ExternalOutput")

        import contextlib
        with contextlib.ExitStack() as _stk:
            wpool = _stk.enter_context(tc.tile_pool(name="wpool", bufs=1))
            apool = _stk.enter_context(tc.tile_pool(name="apool", bufs=1))
            hpool = _stk.enter_context(tc.tile_pool(name="hpool", bufs=1))
            opool = _stk.enter_context(tc.tile_pool(name="opool", bufs=4))
            ppool = _stk.enter_context(tc.tile_pool(name="ppool", bufs=8, space="PSUM"))

            w1_sb = wpool.tile([P, ND * D], bf16, tag="w1sb")
            w2_sb = wpool.tile([P, ND * D], bf16, tag="w2sb")
            xt0_sb = apool.tile([P, ND * TOK_TILE], bf16, tag="xt0")
            xtr_sb = (apool.tile([P, ND * rest], bf16, tag="xtr", name="xtr_sb")
                      if rest else None)
            ht_sbs = [hpool.tile([P, ND * TOK_TILE], bf16, tag=f"ht{j}",
                                 name=f"ht_sb{j}")
                      for j in range(ntile)]

            # PE clock warmup: the HAM throttles a cold PE until it has been
            # busy ~3us. These dummy matmuls have no DMA inputs, so they run
            # right after the startup barrier and un-throttle the PE before
            # the first real matmul's data lands.
            warm_src = opool.tile([P, P], bf16, tag="warm")
            nc.gpsimd.memset(warm_src[:], 0.0)
            warm_ps = ppool.tile([P, TOK_TILE], f32, tag="ps", name="warm_ps")
            for _ in range(12):
                nc.tensor.matmul(warm_ps[:, :P], lhsT=warm_src[:],
                                 rhs=warm_src[:], start=True, stop=True)

            # Input DMA triggers cost ~0.65us each on their queue, so they are
            # split across two queues in strict need-order:
            #   sync:   w1 d-blocks, then xt tiles j>=1, then w2 d-blocks
            #   gpsimd: xt j0 d-blocks (concurrent with w1 so the head of the
            #           pipeline gets its first matmul inputs ~1us sooner)
            # Output stores ride the gpsimd queue (idle once xt is loaded).
            for d in range(ND):
                nc.sync.dma_start(out=w1_sb[:, d * D:(d + 1) * D],
                                  in_=w1t[d * P:(d + 1) * P, :])
                nc.gpsimd.dma_start(out=xt0_sb[:, d * TOK_TILE: d * TOK_TILE + n0],
                                    in_=xt[d * P:(d + 1) * P, 0:n0])
            if rest:
                for d in range(ND):
                    nc.sync.dma_start(out=xtr_sb[:, d * rest:(d + 1) * rest],
                                      in_=xt[d * P:(d + 1) * P, n0:C])
            for d in range(ND):
                nc.sync.dma_start(out=w2_sb[:, d * D:(d + 1) * D],
                                  in_=w2t[d * P:(d + 1) * P, :])

            # Phase 1 — layer 1 for every token tile (consumes only w1 + xt).
            # j=0 runs contraction-major (d outer, 8 PSUM groups in flight) so
            # the PE starts as soon as the first w1/xt blocks land and trickles
            # at DMA rate; later tiles run o-major so relu evictions pipeline.
            pss0 = [ppool.tile([P, TOK_TILE], f32, tag="ps", name=f"ps0_{o}")
                    for o in range(ND)]
            for d in range(ND):
                for o in range(ND):
                    nc.tensor.matmul(
                        pss0[o][:, :n0],
                        lhsT=w1_sb[:, d * D + o * P: d * D + (o + 1) * P],
                        rhs=xt0_sb[:, d * TOK_TILE: d * TOK_TILE + n0],
                        start=(d == 0), stop=(d == ND - 1))
            for o in range(ND):
                nc.scalar.activation(
                    ht_sbs[0][:, o * TOK_TILE: o * TOK_TILE + n0],
                    pss0[o][:, :n0], Relu)

            for j in range(1, ntile):
                n = sizes[j]
                base = j * TOK_TILE - n0  # column offset inside each xtr d-block
                for o in range(ND):
                    ps = ppool.tile([P, TOK_TILE], f32, tag="ps")
                    for d in range(ND):
                        nc.tensor.matmul(
                            ps[:, :n],
                            lhsT=w1_sb[:, d * D + o * P: d * D + (o + 1) * P],
                            rhs=xtr_sb[:, d * rest + base: d * rest + base + n],
                            start=(d == 0), stop=(d == ND - 1))
                    nc.scalar.activation(
                        ht_sbs[j][:, o * TOK_TILE: o * TOK_TILE + n],
                        ps[:, :n], Relu)

            # Phase 2 — layer 2 (w2 is long resident by now). j outer so the
            # final drain after the very last matmul is a single small
            # relu + bf16 store of the short last tile.
            for j in range(ntile):
                n = sizes[j]
                ht_sb = ht_sbs[j]
                for p_ in range(ND):
                    ps2 = ppool.tile([P, TOK_TILE], f32, tag="ps")
                    for o in range(ND):
                        nc.tensor.matmul(
                            ps2[:, :n],
                            lhsT=w2_sb[:, o * D + p_ * P: o * D + (p_ + 1) * P],
                            rhs=ht_sb[:, o * TOK_TILE: o * TOK_TILE + n],
                            start=(o == 0), stop=(o == ND - 1))
                    yo = opool.tile([P, TOK_TILE], bf16, tag="yo")
                    nc.scalar.activation(yo[:, :n], ps2[:, :n], Relu)
                    nc.gpsimd.dma_start(
                        out=yt[p_ * P:(p_ + 1) * P, j * TOK_TILE: j * TOK_TILE + n],
                        in_=yo[:, :n])

    nc.compile()
    return nc


def _get_bass(C: int):
    if C not in _kernel_cache:
        _kernel_cache[C] = _build_bass(C)
    return _kernel_cache[C]


LAST_RESULTS = None  # BassKernelResults of the most recent run (for test harness)


def kernel(x, flat_expert_indices, flat_expert_weights, W1, W2):
    global LAST_RESULTS
    from concourse.bass_utils import run_bass_kernel_spmd
    import ml_dtypes
    bf = ml_dtypes.bfloat16

    x = np.ascontiguousarray(np.asarray(x, dtype=np.float32))
    idx = np.asarray(flat_expert_indices).astype(np.int64)
    w = np.asarray(flat_expert_weights, dtype=np.float32)
    W1 = np.asarray(W1, dtype=np.float32)
    W2 = np.asarray(W2, dtype=np.float32)

    # Dedup (token, expert) pairs: a token whose two top-k slots picked the
    # same expert is computed once with the combined weight. Entry list =
    # [slot-0 entry for every token] + [slot-1 entry for non-dup tokens].
    idx2 = idx.reshape(T, K)
    w2d = w.reshape(T, K)
    dup = idx2[:, 0] == idx2[:, 1]
    tokA = np.arange(T, dtype=np.int64)
    tokB = tokA[~dup]
    toks = np.concatenate([tokA, tokB])
    es = np.concatenate([idx2[:, 0], idx2[~dup, 1]])
    ws = np.concatenate([w2d[:, 0] + np.where(dup, w2d[:, 1], 0.0), w2d[~dup, 1]])
    nent = len(toks)

    order = np.argsort(es, kind="stable")
    counts = np.bincount(es, minlength=E)
    starts = np.zeros(E + 1, dtype=np.int64)
    starts[1:] = np.cumsum(counts)

    # Device capacity per expert: the balanced load (nent/E rounded to 128).
    # The handful of entries routed beyond it are computed on the host —
    # <1% of the work for the target routing distribution.
    C = int(max(TOK_TILE, min(CAP, ((counts.max() + P - 1) // P) * P)))
    nc = _get_bass(C)

    in_maps = []
    sel_list = []
    over_list = []
    for e in range(E):
        pos = order[starts[e]:starts[e + 1]]
        sel_list.append(pos[:C])
        over_list.append(pos[C:])
        te = toks[pos[:C]]
        xte = np.zeros((D, C), dtype=bf)
        if len(te):
            xte[:, :len(te)] = x[te].T.astype(bf)
        in_maps.append({"xt": xte,
                        "w1t": np.ascontiguousarray(W1[e].T).astype(bf),
                        "w2t": np.ascontiguousarray(W2[e].T).astype(bf)})

    trace = bool(os.environ.get("MOE_TRACE"))
    try:
        res = run_bass_kernel_spmd(
            nc, in_maps, list(range(NCORES)),
            trace=trace,
            trace_cores=(list(range(NCORES)) if os.environ.get("MOE_TRACE_MULTI") else [0]) if trace else None,
        )
    except Exception:
        if os.environ.get("MOE_TRACE_STRICT"):
            raise
        # Trace/profiling plumbing can be absent in some environments —
        # fall back to a plain (untraced) run rather than failing.
        prev = os.environ.get("BASS_NEVER_TRACE")
        os.environ["BASS_NEVER_TRACE"] = "1"
        try:
            res = run_bass_kernel_spmd(nc, in_maps, list(range(NCORES)))
        finally:
            if prev is None:
                os.environ.pop("BASS_NEVER_TRACE", None)
            else:
                os.environ["BASS_NEVER_TRACE"] = prev
    LAST_RESULTS = res

    yent = np.zeros((nent, D), dtype=np.float32)
    for e in range(E):
        sel = sel_list[e]
        if len(sel):
            yent[sel] = res.results[e]["yt"][:, :len(sel)].T.astype(np.float32)
        over = over_list[e]
        if len(over):
            h = np.maximum(x[toks[over]] @ W1[e].T, 0.0)
            yent[over] = np.maximum(h @ W2[e].T, 0.0)

    contrib = yent * ws[:, None]
    out = contrib[:T].copy()
    out[tokB] += contrib[T:]
    return out.astype(np.float32)
